# revision 15
# baseline (speedup 1.0000x reference)
"""AConvCircular3D kernel for 8 trn2 NeuronCores.

Sharding: core i handles (batch b = i//4, head h = i%4).
Each core computes, for its (b, h):
  - 3x3x3 circular conv of x[b] producing 32 channels:
      rows 0-7:   q_h (scale folded into weights)
      rows 8-15:  k_h
      rows 16-23: v_h
      rows 24-31: its 8 channels of the 32-channel "init" conv output
  - full softmax attention for head h (N=4096 keys/queries, dk=dv=8)
  - the torch-faithful reshape (b,H,N,dv)->(b,H*dv,spatial) and a partial
    1x1 conv:  pout = W_out[:, 8h:8h+8] @ comb_h   (32 x 4096)
Host sums pout over the 4 head-cores of each batch, adds bias, and
concatenates with the gathered init channels.
"""
import os
import sys

for _p in ("/opt/trn_rl_repo", "/root/.axon_site/_ro/trn_rl_repo"):
    if os.path.isdir(_p) and _p not in sys.path:
        sys.path.insert(0, _p)

import numpy as np

NUM_HEADS = 4
DKH = 8
DVH = 8
IN_CH = 32
S = 16
NSP = S * S * S           # 4096 spatial positions
PADW = S + 2              # 18
PADV = PADW ** 3          # 5832
M_TILE = 128              # keys per tile
N_MTILES = NSP // M_TILE  # 32

_CACHE = {}


def _build(n_cores=8, m_tiles=N_MTILES, debug=()):
    import concourse.bass as bass
    import concourse.mybir as mybir
    import concourse.tile as tile
    from concourse.tile import add_dep_helper
    from concourse import bacc
    from concourse.bass import ts
    from concourse.masks import make_identity

    BF16 = mybir.dt.bfloat16
    F32 = mybir.dt.float32
    F32R = mybir.dt.float32r
    EXP = mybir.ActivationFunctionType.Exp
    COPY = mybir.ActivationFunctionType.Copy

    nc = bacc.Bacc("TRN2", target_bir_lowering=False, debug=False,
                   num_devices=n_cores)

    xpad = nc.declare_dram_parameter("xpad", [IN_CH, PADV], F32R, isOutput=False)
    wcv = nc.declare_dram_parameter("wcv", [96, 9 * 32], F32R, isOutput=False)
    wout = nc.declare_dram_parameter("wout", [8, 32], F32, isOutput=False)
    iout = nc.declare_dram_parameter("iout", [8, NSP], F32R, isOutput=True)
    pout = nc.declare_dram_parameter("pout", [32, NSP], F32, isOutput=True)

    if "stg32" in debug:
        d_stg32 = nc.declare_dram_parameter("d_stg32", [32, NSP], F32R, isOutput=True)
    if "v" in debug:
        d_v = nc.declare_dram_parameter("d_v", [9, NSP], BF16, isOutput=True)
    if "vt" in debug:
        d_vt = nc.declare_dram_parameter("d_vt", [128, N_MTILES * 9], BF16, isOutput=True)
    if "et0" in debug:
        d_et0 = nc.declare_dram_parameter("d_et0", [128, NSP], BF16, isOutput=True)
    if "av" in debug:
        d_av = nc.declare_dram_parameter("d_av", [128, 32 * 9], F32, isOutput=True)
    if "a" in debug:
        d_a = nc.declare_dram_parameter("d_a", [128, 32 * 8], BF16, isOutput=True)
        d_a32 = nc.declare_dram_parameter("d_a32", [128, 32 * 8], F32, isOutput=True)
        d_avf = nc.declare_dram_parameter("d_avf", [128, 32 * 9], F32, isOutput=True)
        d_rcp = nc.declare_dram_parameter("d_rcp", [128, 32], F32, isOutput=True)
    if "comb" in debug:
        d_comb = nc.declare_dram_parameter("d_comb", [8, NSP], BF16, isOutput=True)
    if "k" in debug:
        d_k = nc.declare_dram_parameter("d_k", [8, NSP], F32R, isOutput=True)
    if "lg0" in debug:
        d_lg0 = nc.declare_dram_parameter("d_lg0", [128, 1024], F32, isOutput=True)

    with tile.TileContext(nc) as tc:
        with tc.tile_pool(name="sb", bufs=1) as sb, \
             tc.tile_pool(name="et", bufs=3) as etp, \
             tc.tile_pool(name="dr", bufs=1, space="DRAM") as drp:
            a_bounce = drp.tile([NSP, DVH], F32)

            # ---- stage padded x, replicated 3x with dx shifts (partitions 32g) ----
            xp = sb.tile([96, PADV], F32R)
            for g in range(3):
                n = PADV - g
                for piece in range(4):
                    lo = (n * piece) // 4
                    hi = (n * (piece + 1)) // 4
                    nc.sync.dma_start(out=xp[32 * g:32 * g + 32, lo:hi],
                                      in_=xpad[:, g + lo:g + hi])
            xp4 = xp[:].rearrange("p (z y x) -> p z y x", z=PADW, y=PADW, x=PADW)

            w_sb = sb.tile([96, 9 * 32], F32R)
            nc.sync.dma_start(out=w_sb[:], in_=wcv[:])

            wout_f = sb.tile([8, 32], F32)
            nc.sync.dma_start(out=wout_f[:], in_=wout[:])
            wout_b = sb.tile([8, 32], BF16)
            nc.vector.tensor_copy(out=wout_b[:], in_=wout_f[:])

            ident = sb.tile([128, 128], BF16)
            make_identity(nc, ident)

            # ---- conv: 9 rounds (dz,dy) x 8 chunks (z pairs), K=96, f32r ----
            with tc.tile_pool(name="cv", bufs=1, space="PSUM") as cvp:
                cv = cvp.tile([32, NSP], F32)
                for c in range(8):
                    for r in range(9):
                        dz, dy = r // 3, r % 3
                        rhs = xp4[:, 2 * c + dz:2 * c + dz + 2,
                                  dy:dy + S, 0:S]
                        nc.tensor.matmul(cv[:, ts(c, 512)],
                                         w_sb[:, ts(r, 32)],
                                         rhs, start=(r == 0), stop=(r == 8))
                # evict: f32 copy (q,k,init) on ACT; bf16 copy (v) on DVE
                stg32 = sb.tile([32, NSP], F32R)
                stg16 = sb.tile([32, NSP], BF16)
                nc.scalar.activation(out=stg32[:], in_=cv[:], func=COPY)
                nc.vector.tensor_copy(out=stg16[:], in_=cv[:])

            # init partial goes straight out
            for piece in range(2):
                nc.sync.dma_start(out=iout[:, ts(piece, NSP // 2)],
                                  in_=stg32[24:32, ts(piece, NSP // 2)])
            # K needs to be lhsT at partition base 0: remap via DMA
            k_sb = sb.tile([8, NSP], F32R)
            for piece in range(2):
                nc.sync.dma_start(out=k_sb[:, ts(piece, NSP // 2)],
                                  in_=stg32[8:16, ts(piece, NSP // 2)])
            # V' = [V; ones] at base 0, bf16: memset all to 1.0 first (engine
            # ops cannot start at partition 8), then overwrite rows 0-7 via DMA
            v_sb = sb.tile([9, NSP], BF16)
            nc.vector.memset(v_sb[:], 1.0)
            for piece in range(2):
                nc.sync.dma_start(out=v_sb[0:8, ts(piece, NSP // 2)],
                                  in_=stg16[16:24, ts(piece, NSP // 2)])

            q_ap = stg32[0:8, :]

            # ---- VT' tiles: transpose V' [9,128] chunks -> [128, t, 9] ----
            vt = sb.tile([128, m_tiles, 9], BF16)
            with tc.tile_pool(name="tp", bufs=2, space="PSUM") as tpp:
                for t in range(m_tiles):
                    tp = tpp.tile([128, 9], BF16)
                    nc.tensor.transpose(tp[:], v_sb[:, ts(t, M_TILE)],
                                        ident[0:9, 0:9])
                    nc.vector.tensor_copy(out=vt[:, t, :], in_=tp[:])

            # ---- attention m-loop ----
            zc = sb.tile([1, 128], BF16)
            zr = sb.tile([1, 32 * 9], BF16)
            nc.vector.memset(zc[:], 0.0)
            nc.vector.memset(zr[:], 0.0)
            with tc.tile_pool(name="av", bufs=1, space="PSUM") as avp, \
                 tc.tile_pool(name="lg", bufs=2, space="PSUM") as lgp:
                av = avp.tile([128, 32 * 9], F32)
                # zero the whole accumulator with a start=True matmul: clears
                # stale has_written bits bank-wide AND creates an AP-overlap
                # dependency so no accumulating matmul can be scheduled first
                nc.tensor.matmul(av[:], zc[:], zr[:], start=True, stop=False)
                for t in range(m_tiles):
                    et = etp.tile([128, NSP], BF16)
                    for qq in range(4):
                        lg = lgp.tile([128, 1024], F32)
                        for hh in range(2):
                            nq0 = qq * 1024 + hh * 512
                            nc.tensor.matmul(
                                lg[:, ts(hh, 512)],
                                k_sb[:, ts(t, M_TILE)],
                                q_ap[:, nq0:nq0 + 512],
                                start=True, stop=True)
                        if "lg0" in debug and t == 0 and qq == 0:
                            lg_dump = sb.tile([128, 1024], F32)
                            nc.vector.tensor_copy(out=lg_dump[:], in_=lg[:])
                            nc.sync.dma_start(out=d_lg0[:], in_=lg_dump[:])
                        nc.scalar.activation(out=et[:, ts(qq, 1024)],
                                             in_=lg[:], func=EXP)
                    for j in range(32):
                        nc.tensor.matmul(av[:, ts(j, 9)],
                                         et[:, ts(j, M_TILE)],
                                         vt[:, t, :],
                                         start=False,
                                         stop=(t == m_tiles - 1))
                    if "et0" in debug and t == 0:
                        nc.sync.dma_start(out=d_et0[:], in_=et[:])

                if "av" in debug:
                    av_dump = sb.tile([128, 32 * 9], F32)
                    nc.vector.tensor_copy(out=av_dump[:], in_=av[:])
                    nc.sync.dma_start(out=d_av[:], in_=av_dump[:])
                # ---- normalize: A = num/den ----
                # (avoid strided-PSUM reads and per-partition-scalar ops: full
                # flat copy to SBUF first, then broadcast tensor_tensor mul)
                avf = sb.tile([128, 32 * 9], F32)
                nc.vector.tensor_copy(out=avf[:], in_=av[:])
                avf3 = avf[:].rearrange("p (j v) -> p j v", v=9)
                rcp = sb.tile([128, 32], F32)
                nc.vector.reciprocal(out=rcp[:], in_=avf3[:, :, 8])
                rcp_b = rcp[:].to_broadcast([128, 32, 8])
                a_sb = sb.tile([128, 32, 8], F32)
                nc.vector.tensor_mul(a_sb[:], avf3[:, :, 0:8], rcp_b)

            # ---- funky reshape via DRAM bounce ----
            wdma = nc.sync.dma_start(
                out=a_bounce[:].rearrange("(c p) v -> p c v", p=128),
                in_=a_sb[:])
            comb = sb.tile([8, NSP], BF16)
            rdma = nc.gpsimd.dma_start(
                out=comb[:],
                in_=a_bounce[:].rearrange("(c r) v -> c (r v)", c=8))
            add_dep_helper(rdma.ins, wdma.ins,
                           reason="a_bounce dram RAW: read-back after write")
            if "stg32" in debug:
                nc.sync.dma_start(out=d_stg32[:], in_=stg32[:])
            if "v" in debug:
                nc.sync.dma_start(out=d_v[:], in_=v_sb[:])
            if "vt" in debug:
                nc.sync.dma_start(out=d_vt[:], in_=vt[:].rearrange("p t v -> p (t v)"))
            if "a" in debug:
                nc.sync.dma_start(out=d_a[:], in_=a_sb[:].rearrange("p j v -> p (j v)"))
                a32 = sb.tile([128, 32 * 8], F32)
                nc.vector.tensor_copy(out=a32[:], in_=a_sb[:].rearrange("p j v -> p (j v)"))
                nc.sync.dma_start(out=d_a32[:], in_=a32[:])
                nc.sync.dma_start(out=d_avf[:], in_=avf[:])
                nc.sync.dma_start(out=d_rcp[:], in_=rcp[:])
            if "comb" in debug:
                nc.sync.dma_start(out=d_comb[:], in_=comb[:])
            if "k" in debug:
                nc.sync.dma_start(out=d_k[:], in_=k_sb[:])

            # ---- partial 1x1 conv ----
            with tc.tile_pool(name="po", bufs=1, space="PSUM") as pop:
                po = pop.tile([32, NSP], F32)
                for c in range(8):
                    nc.tensor.matmul(po[:, ts(c, 512)], wout_b[:],
                                     comb[:, ts(c, 512)], start=True, stop=True)
                po_sb = sb.tile([32, NSP], F32)
                nc.scalar.activation(out=po_sb[:], in_=po[:], func=COPY)
            for piece in range(2):
                nc.sync.dma_start(out=pout[:, ts(piece, NSP // 2)],
                                  in_=po_sb[:, ts(piece, NSP // 2)])

    nc.compile()
    return nc


def _get_nc(n_cores=8, m_tiles=N_MTILES):
    key = (n_cores, m_tiles)
    if key not in _CACHE:
        _CACHE[key] = _build(n_cores, m_tiles)
    return _CACHE[key]


def _host_inputs(x, w_init, w_qkv, w_out):
    """Build per-core input maps. Core i = (b = i//4, h = i%4)."""
    x = np.asarray(x, dtype=np.float32)
    w_init = np.asarray(w_init, dtype=np.float32)
    w_qkv = np.asarray(w_qkv, dtype=np.float32)
    w_out = np.asarray(w_out, dtype=np.float32)

    xpads = []
    for b in range(2):
        xp = np.pad(x[b], ((0, 0), (1, 1), (1, 1), (1, 1)), mode="wrap")
        xpads.append(np.ascontiguousarray(xp.reshape(IN_CH, PADV)))

    in_maps = []
    scale = DKH ** -0.5
    for i in range(8):
        b, h = i // 4, i % 4
        # conv weight rows: [q(8) k(8) v(8) init(8)] -> (32 oc, 32 ic, 3,3,3)
        W = np.concatenate([
            w_qkv[8 * h:8 * h + 8] * scale,
            w_qkv[32 + 8 * h:32 + 8 * h + 8],
            w_qkv[64 + 8 * h:64 + 8 * h + 8],
            w_init[8 * h:8 * h + 8],
        ], axis=0)  # (32, 32, 3, 3, 3)
        wcv = np.zeros((96, 9, 32), dtype=np.float32)
        for g in range(3):
            for dz in range(3):
                for dy in range(3):
                    wcv[32 * g:32 * g + 32, 3 * dz + dy, :] = W[:, :, dz, dy, g].T
        wout_h = np.ascontiguousarray(w_out[:, 8 * h:8 * h + 8, 0, 0, 0].T)  # (8, 32)
        in_maps.append({
            "xpad": xpads[b],
            "wcv": np.ascontiguousarray(wcv.reshape(96, 9 * 32)),
            "wout": wout_h.astype(np.float32),
        })
    return in_maps


def kernel(x, w_init, w_qkv, w_out, b_out):
    from concourse.bass_utils import run_bass_kernel_spmd

    nc = _get_nc()
    in_maps = _host_inputs(x, w_init, w_qkv, w_out)
    res = run_bass_kernel_spmd(nc, in_maps, core_ids=list(range(8)))

    b_out = np.asarray(b_out, dtype=np.float32)
    out = np.zeros((2, 64, NSP), dtype=np.float32)
    for i in range(8):
        b, h = i // 4, i % 4
        out[b, 8 * h:8 * h + 8] = res.results[i]["iout"]
        out[b, 32:64] += res.results[i]["pout"]
    out[:, 32:64] += b_out[None, :, None]
    return out.reshape(2, 64, S, S, S)
